# revision 29
# baseline (speedup 1.0000x reference)
"""BertSelfAttention (B=4, S=4096, D=512) on 8 TRN2 NeuronCores.

Sharding: core c handles batch b = c//2 and query-row half h = c%2
(2048 q rows). K/V are computed on-core for the full 4096 keys of that
batch (duplicated across the pair), which avoids all collectives.

Layout trick: everything is computed transposed so no on-device
transposes are needed:
  QT[e, q] = Wq @ x.T          (lhsT = WqT chunks, rhs = xT chunks)
  KT[e, k] = Wk @ x.T
  V [k, e] = x @ Wv.T          (lhsT = xT chunks,  rhs = WvT)
  ST[k, q] = K Q.T             (lhsT = KT chunks,  rhs = QT)   -> exp -> PT
  OT[e, q] = V.T P.T           (lhsT = V chunks,   rhs = PT)
Softmax runs without max-subtraction (scores are ~N(0, 0.2^2), so exp
cannot overflow and the result is mathematically identical).

fp8 fast path: the two big matmuls (ST and OT) run in float8e4 with
MatmulPerfMode.DoubleRow (2 contraction chunks per pass, 157 TF/s =
2x bf16). Raw fp8 storage of P = exp(s) ~ 1 would cost ~2% relative
error, so P is centered: p = P - 1 (|p| ~ 0.2) is stored in fp8 and
  P @ V = colsum(V) + p @ V
with colsum(V) = (sum_k x_k) @ Wv.T computed exactly in bf16 (a DVE
free-dim reduction of x plus four 1-column matmuls) and the partner
half's colsum exchanged via a tiny third AllGather. The softmax
denominator Z = 4096 + sum_k p comes from 16 fp8 all-ones DoubleRow
matmuls per q-chunk (an all-ones lhsT replicates the partition
reduction across all 128 output partitions), so no elementwise engine
touches the 8.4M-element rowsum. The p = P - 1 conversion to fp8 is
the only remaining elementwise pass over the scores; it alternates
between the Vector and GpSimd engines per k-tile (each is ~2x slower
than the Scalar engine's exp, so half each keeps the pipeline
exp-rate-limited). Q/K/V projections stay bf16 (fp32 PSUM): V must be
bf16 for the colsum identity to hold, and fp8 Q/K projections would
double quantization noise for only ~14 us.

Scaling: Wq/bq/Wk/bk/Wv are scaled by 16 on the host and p by 16 on
chip (fused into the tensor_scalar), lifting the fp8 operands out of
the e4m3 subnormal range; the exp act scale (SCALE/256), a 16x on the
xsum->bf16 colsum copy, and Z' = 16*rs + 2^20 fold every factor back
out exactly, leaving the (ops + colsum)*recip evacuation unchanged.

Schedule: the score pipeline is exp-paced (~700ns per 128x512 tile on
the Scalar engine vs ~520ns of DoubleRow matmul), so every other PE
consumer is threaded into that slack as emission-order "fillers":
V-projection groups inside the first two local score blocks, the QT
chunks at block boundaries (Vector-engine evacuation, to stay off the
exp stream), and each finish (Z pass + 4 OT accumulation groups)
inside the NEXT q-chunk's remote score block. Only the first remote
block and the last finish run unfilled.

bq/bk are fused into the QT/KT PSUM evacuation as per-partition ACT
biases; bv is added after normalization (exact: P@(V + 1*bv)/rowsum =
P@V/rowsum + bv). The final evacuation fuses (+colsum, *1/Z) into one
scalar_tensor_tensor. A burst of throwaway matmuls warms the PE HAM
clock gate during the initial DMA wait, and input DMAs are
column-chunked and issued in first-consumer order on the SP hardware
queue (weights on the ACT queue) so the first projection starts after
~1 MB of transfer.
"""

import sys

for _p in ("/opt/trn_rl_repo", "/root/.axon_site/_ro/trn_rl_repo"):
    if _p not in sys.path:
        sys.path.append(_p)

import numpy as np
import ml_dtypes

B, S, D = 4, 4096, 512
NCORES = 8
SQ = S // 2  # query rows per core
P = 128
NQ = 512  # q-chunk width (moving free dim)
DT = D // P  # 4 contraction chunks for d
ET = D // P  # 4 e tiles
KTI = S // P  # 32 k tiles
QC = SQ // NQ  # 4 q chunks per core
KC = S // NQ  # 8 k chunks (KT projection)
HKT = KTI // 2  # 16 local k-tiles per core
HS = S // 2  # 2048 local keys per core
SCALE = 1.0 / float(np.sqrt(np.float32(D)))

_CACHE = {}


def _split_excess_waits(nc, mybir, max_waits=1):
    """This walrus build rejects instructions carrying more than a couple of
    sync waits. Cap every instruction at `max_waits`, spilling the rest onto
    same-engine InstNoOps inserted immediately before it (equivalent
    semantics: the engine's stream stalls at the nop instead)."""
    for f in nc.m.functions:
        for bb in f.blocks:
            old = list(bb.instructions)
            new = []
            for inst in old:
                si = inst.sync_info
                waits = list(si.on_wait) if si is not None and si.on_wait else []
                if len(waits) > max_waits:
                    keep = waits[-max_waits:]
                    excess = waits[:-max_waits]
                    for i in range(0, len(excess), max_waits):
                        nop = mybir.InstNoOp(
                            name=f"waitnop-{nc.next_id()}", ins=[], outs=[]
                        )
                        nop.engine = inst.engine
                        nop.sync_info = mybir.SyncInfo(
                            on_wait=excess[i : i + max_waits], on_update=[]
                        )
                        new.append(nop)
                    inst.sync_info = mybir.SyncInfo(
                        on_wait=keep,
                        on_update=list(si.on_update) if si.on_update else [],
                    )
                new.append(inst)
            if len(new) != len(old):
                bb.instructions[:] = new


def _build_nc():
    import concourse.bass as bass
    import concourse.mybir as mybir
    import concourse.tile as tile
    from contextlib import ExitStack

    bf = mybir.dt.bfloat16
    f8 = mybir.dt.float8e4
    f32 = mybir.dt.float32
    AF = mybir.ActivationFunctionType
    ALU = mybir.AluOpType
    DR = mybir.MatmulPerfMode.DoubleRow

    u32 = mybir.dt.uint32
    nc = bass.Bass()
    xT = nc.declare_dram_parameter("xT", [D, SQ], bf, isOutput=False)
    wqT = nc.declare_dram_parameter("wqT", [D, D], bf, isOutput=False)
    wkT = nc.declare_dram_parameter("wkT", [D, D], bf, isOutput=False)
    wvT = nc.declare_dram_parameter("wvT", [D, D], bf, isOutput=False)
    bqp = nc.declare_dram_parameter("bq", [P, ET], f32, isOutput=False)
    bkp = nc.declare_dram_parameter("bk", [P, ET], f32, isOutput=False)
    bvp = nc.declare_dram_parameter("bv", [P, ET], f32, isOutput=False)
    # Host-computed row bases into the AllGather outputs for the PARTNER
    # half (rank-dependent: (1-h)*512 + e*128 for KT, (1-h)*2048 + j*128
    # for V). Drives dynamic (register-offset) DMAs.
    poffp = nc.declare_dram_parameter("poff", [1, 2], u32, isOutput=False)
    ot = nc.declare_dram_parameter("ot", [D, SQ], f32, isOutput=True)

    with tile.TileContext(nc) as tc, ExitStack() as ctx:
        const_pool = ctx.enter_context(tc.tile_pool(name="const", bufs=1))
        persist = ctx.enter_context(tc.tile_pool(name="persist", bufs=1))
        outp = ctx.enter_context(tc.tile_pool(name="outp", bufs=2))

        ones = const_pool.tile([P, P], bf, tag="ones")
        nc.vector.memset(ones, 1.0)
        ones8 = const_pool.tile([P, 2, P], f8, tag="ones8")
        nc.vector.memset(ones8, 1.0)
        bq_sb = const_pool.tile([P, ET], f32, tag="bq")
        bk_sb = const_pool.tile([P, ET], f32, tag="bk")
        bv_sb = const_pool.tile([P, ET], f32, tag="bv")
        wq_sb = [const_pool.tile([P, D], bf, tag=f"wq{d}", name=f"wq{d}") for d in range(DT)]
        wk_sb = [const_pool.tile([P, D], bf, tag=f"wk{d}", name=f"wk{d}") for d in range(DT)]
        wv_sb = [const_pool.tile([P, D], bf, tag=f"wv{d}", name=f"wv{d}") for d in range(DT)]
        # Q/K/V in fp8 for the DoubleRow matmuls. Middle dim indexes the
        # 128-row contraction chunk so a [:, 2i:2i+2, :] slice yields the
        # DoubleRow pair.
        qt_sb = persist.tile([P, ET, SQ], f8, tag="qt", name="qt")
        # K/V k-order per core: [my half, partner half]. Separate tiles per
        # half so partner DMA-writes create no false deps on local reads.
        kt_loc = persist.tile([P, ET, HS], f8, tag="ktl", name="ktl")
        kt_rem = persist.tile([P, ET, HS], f8, tag="ktr", name="ktr")
        v_loc = persist.tile([P, HKT, D], f8, tag="vl", name="vl")
        v_rem = persist.tile([P, HKT, D], f8, tag="vr", name="vr")
        poff_sb = const_pool.tile([1, 2], mybir.dt.uint32, tag="poff")
        # colsum(V) pieces: DVE x-rowsum -> tiny matmuls -> AllGather.
        xsum_f = persist.tile([P, DT], f32, tag="xsf", name="xsf")
        xsum_b = persist.tile([P, DT], bf, tag="xsb", name="xsb")
        cs_loc = persist.tile([1, D], f32, tag="csl", name="csl")
        cs2_sb = persist.tile([P, 2, ET], f32, tag="cs2", name="cs2")
        colsum_sb = persist.tile([P, ET], f32, tag="cs", name="cs")

        # ---- One merged region: projections are interleaved into the
        # local-half score blocks as PE fillers (the score pipeline is
        # exp-paced, leaving ~25% PE slack per tile) ----
        with (
            tc.tile_pool(name="xin", bufs=1) as xin_pool,
            tc.tile_pool(name="psA", bufs=2, space="PSUM") as psA,
            tc.tile_pool(name="dram", bufs=1, space="DRAM") as dram,
            tc.tile_pool(name="pt", bufs=1) as pt_pool,
            tc.tile_pool(name="pbf", bufs=6) as pbf_pool,
            tc.tile_pool(name="ps_st", bufs=3, space="PSUM") as ps_st,
            tc.tile_pool(name="ps_sum", bufs=1, space="PSUM") as ps_sum,
            tc.tile_pool(name="ps_ot", bufs=2, space="PSUM") as ps_ot,
        ):
            ktl_d = dram.tile([ET * P, HS], f8, tag="ktl_d")
            ktg_d = dram.tile([2 * ET * P, HS], f8, tag="ktg_d")
            vl_d = dram.tile([HKT * P, D], f8, tag="vl_d")
            vg_d = dram.tile([2 * HKT * P, D], f8, tag="vg_d")
            csl_d = dram.tile([1, D], f32, tag="csl_d")
            csg_d = dram.tile([2, D], f32, tag="csg_d")

            x_sb = [xin_pool.tile([P, HS], bf, tag=f"x{d}", name=f"x{d}") for d in range(DT)]
            # Column-chunked loads in first-consumer order (SP HW queue);
            # weights ride the ACT HW queue in parallel.
            # One narrow chunk first (fast start for the first KT groups),
            # then the remaining columns in one wide DMA per d-tile to keep
            # SP-side issue overhead (~0.6us per DMA) off the critical path.
            for d in range(DT):
                nc.sync.dma_start(
                    out=x_sb[d][:, :NQ], in_=xT[d * P : (d + 1) * P, :NQ]
                )
            nc.sync.dma_start(out=bk_sb, in_=bkp[:, :])
            nc.sync.dma_start(out=bq_sb, in_=bqp[:, :])
            nc.sync.dma_start(out=bv_sb, in_=bvp[:, :])
            nc.sync.dma_start(out=poff_sb, in_=poffp[:, :])
            for kc in range(1, QC):
                for d in range(DT):
                    nc.sync.dma_start(
                        out=x_sb[d][:, kc * NQ : (kc + 1) * NQ],
                        in_=xT[d * P : (d + 1) * P, kc * NQ : (kc + 1) * NQ],
                    )
            for d in range(DT):
                nc.scalar.dma_start(out=wk_sb[d], in_=wkT[d * P : (d + 1) * P, :])
            for d in range(DT):
                nc.scalar.dma_start(out=wv_sb[d], in_=wvT[d * P : (d + 1) * P, :])
            for d in range(DT):
                nc.scalar.dma_start(out=wq_sb[d], in_=wqT[d * P : (d + 1) * P, :])

            # Warm the PE HAM clock gate (~3.4us of activity flips it from
            # 1.2 to 2.4 GHz) with throwaway matmuls while the first input
            # DMAs are still in flight. (PSUM is fully subscribed, so the
            # warm target is a slice of a rotating projection-pool tile.)
            warm_ps = psA.tile([P, NQ], f32, tag="ps", name="warm_ps")
            for _ in range(40):
                nc.tensor.matmul(
                    warm_ps[:, :P], lhsT=ones, rhs=ones, start=True, stop=True
                )

            # KT local half [e, 0:2048] (bias bk fused on evacuation)
            for kc in range(QC):
                for e in range(ET):
                    ps = psA.tile([P, NQ], f32, tag="ps")
                    for d in range(DT):
                        nc.tensor.matmul(
                            ps,
                            lhsT=wk_sb[d][:, e * P : (e + 1) * P],
                            rhs=x_sb[d][:, kc * NQ : (kc + 1) * NQ],
                            start=(d == 0),
                            stop=(d == DT - 1),
                        )
                    # KT evac on the (idle-in-phase-1) Vector engine keeps the
                    # Scalar engine free for the exp stream later. (GpSimd
                    # cannot read PSUM on this hardware.)
                    nc.vector.tensor_scalar(
                        out=kt_loc[:, e, kc * NQ : (kc + 1) * NQ],
                        in0=ps,
                        scalar1=bk_sb[:, e : e + 1],
                        scalar2=None,
                        op0=mybir.AluOpType.add,
                    )
            for e in range(ET):
                nc.sync.dma_start(out=ktl_d[e * P : (e + 1) * P, :], in_=kt_loc[:, e, :])

            pairs = [[2 * i, 2 * i + 1] for i in range(NCORES // 2)]
            # KT gather goes out as early as possible: the remote-half score
            # blocks are the first consumers of partner data.
            nc.gpsimd.collective_compute(
                "AllGather",
                mybir.AluOpType.bypass,
                replica_groups=pairs,
                ins=[ktl_d.opt()],
                outs=[ktg_d.opt()],
            )
            SP = [mybir.EngineType.SP]
            kt_base = nc.values_load(
                poff_sb[0:1, 0:1], engines=SP,
                min_val=0, max_val=ET * P,
                skip_runtime_bounds_check=True,
            )
            nc.sync.dma_start(
                out=kt_rem,
                in_=ktg_d[bass.ds(kt_base, ET * P), :].rearrange(
                    "(e p) c -> p e c", p=P
                ),
            )

            # colsum(V) local half = (sum_k x) @ Wv.T, exact in bf16.
            for d in range(DT):
                nc.vector.tensor_reduce(
                    out=xsum_f[:, d : d + 1],
                    in_=x_sb[d],
                    axis=mybir.AxisListType.X,
                    op=ALU.add,
                )
            # x16 here makes colsum = 256 * colsum_true, matching ops =
            # (16p) @ (16V) = 256 * p@V from the scaled weights below.
            nc.scalar.mul(out=xsum_b, in_=xsum_f, mul=16.0)
            cs_ps = psA.tile([P, NQ], f32, tag="ps", name="cs_ps")
            for d in range(DT):
                nc.tensor.matmul(
                    cs_ps[0:1, :],
                    lhsT=xsum_b[:, d : d + 1],
                    rhs=wv_sb[d][:, :],
                    start=(d == 0),
                    stop=(d == DT - 1),
                )
            nc.scalar.copy(out=cs_loc, in_=cs_ps[0:1, :])
            nc.sync.dma_start(out=csl_d, in_=cs_loc)

            # V projection groups, emitted later as PE fillers inside the
            # exp-paced local score blocks (no bias; bv folded in at the
            # end). Evacuation on the Vector engine.
            def v_group(k):
                ps = psA.tile([P, D], f32, tag="ps", name="v_ps")
                for d in range(DT):
                    nc.tensor.matmul(
                        ps,
                        lhsT=x_sb[d][:, k * P : (k + 1) * P],
                        rhs=wv_sb[d][:, :],
                        start=(d == 0),
                        stop=(d == DT - 1),
                    )
                nc.vector.tensor_copy(out=v_loc[:, k, :], in_=ps)
                nc.sync.dma_start(out=vl_d[k * P : (k + 1) * P, :], in_=v_loc[:, k, :])

            def v_exchange():
                # Exchange the V half + colsum; remote halves are needed
                # only from finish(0) onward.
                nc.gpsimd.collective_compute(
                    "AllGather",
                    mybir.AluOpType.bypass,
                    replica_groups=pairs,
                    ins=[vl_d.opt()],
                    outs=[vg_d.opt()],
                )
                v_base = nc.values_load(
                    poff_sb[0:1, 1:2], engines=SP,
                    min_val=0, max_val=HKT * P,
                    skip_runtime_bounds_check=True,
                )
                nc.sync.dma_start(
                    out=v_rem,
                    in_=vg_d[bass.ds(v_base, HKT * P), :].rearrange(
                        "(j p) c -> p j c", p=P
                    ),
                )
                nc.gpsimd.collective_compute(
                    "AllGather",
                    mybir.AluOpType.bypass,
                    replica_groups=pairs,
                    ins=[csl_d.opt()],
                    outs=[csg_d.opt()],
                )
                # Both ranks' colsum halves: transpose-load then add (order
                # of the halves is irrelevant to the sum).
                nc.sync.dma_start(
                    out=cs2_sb,
                    in_=csg_d.rearrange("r (e p) -> p r e", p=P),
                )
                nc.vector.tensor_add(colsum_sb, cs2_sb[:, 0, :], cs2_sb[:, 1, :])

            def qt_group(qc, on_scalar):
                # QT[e, q] (bias bq fused on evacuation). Evacuation goes to
                # the Scalar engine only before the exp stream starts.
                qsl = slice(qc * NQ, (qc + 1) * NQ)
                for e in range(ET):
                    ps = psA.tile([P, NQ], f32, tag="ps", name="qt_ps")
                    for d in range(DT):
                        nc.tensor.matmul(
                            ps,
                            lhsT=wq_sb[d][:, e * P : (e + 1) * P],
                            rhs=x_sb[d][:, qsl],
                            start=(d == 0),
                            stop=(d == DT - 1),
                        )
                    if on_scalar:
                        nc.scalar.activation(
                            out=qt_sb[:, e, qsl],
                            in_=ps,
                            func=AF.Identity,
                            bias=bq_sb[:, e : e + 1],
                            scale=1.0,
                        )
                    else:
                        nc.vector.tensor_scalar(
                            out=qt_sb[:, e, qsl],
                            in0=ps,
                            scalar1=bq_sb[:, e : e + 1],
                            scalar2=None,
                            op0=mybir.AluOpType.add,
                        )

            ptl_tiles = {}
            ptp_tiles = {}

            def st_block(qc, k0, k1, fillers=()):
                fillers = list(fillers)
                qsl = slice(qc * NQ, (qc + 1) * NQ)
                if k0 == 0:
                    ptl_tiles[qc] = pt_pool.tile(
                        [P, HKT, NQ], f8, tag="ptl", name=f"ptl{qc}", bufs=4
                    )
                else:
                    ptp_tiles[qc] = pt_pool.tile(
                        [P, HKT, NQ], f8, tag="ptp", name=f"ptp{qc}", bufs=2
                    )
                for k in range(k0, k1):
                    ps = ps_st.tile([P, NQ], f32, tag="st", name="st_ps")
                    if k < HKT:
                        kt, kk = kt_loc, k
                    else:
                        kt, kk = kt_rem, k - HKT
                    for i in range(ET // 2):
                        nc.tensor.matmul(
                            ps,
                            lhsT=kt[:, 2 * i : 2 * i + 2, kk * P : (kk + 1) * P],
                            rhs=qt_sb[:, 2 * i : 2 * i + 2, qsl],
                            start=(i == 0),
                            stop=(i == ET // 2 - 1),
                            perf_mode=DR,
                        )
                    pbf = pbf_pool.tile([P, NQ], bf, tag="pbf", name="pbf")
                    nc.scalar.activation(
                        out=pbf, in_=ps, func=AF.Exp, scale=SCALE / 256.0
                    )
                    if k < HKT:
                        p8 = ptl_tiles[qc][:, k, :]
                    else:
                        p8 = ptp_tiles[qc][:, k - HKT, :]
                    # p8 = (exp(s) - 1) * 16; alternate engines so neither
                    # becomes the pipeline rate limiter.
                    eng = nc.vector if (k % 2 == 0) else nc.gpsimd
                    eng.tensor_scalar(
                        out=p8,
                        in0=pbf,
                        scalar1=-1.0,
                        scalar2=16.0,
                        op0=mybir.AluOpType.add,
                        op1=mybir.AluOpType.mult,
                    )
                    if fillers and k % 2 == 1:
                        fillers.pop(0)()
                for fill in fillers:
                    fill()

            def z_group(qc):
                # Z via fp8 all-ones DoubleRow matmuls over the p8 tiles:
                # out[m, q] = sum_pair p8[k, q] for every partition m, i.e.
                # the partition reduction arrives already replicated.
                rs_ps = ps_sum.tile([P, NQ], f32, tag="rs")
                for j in range(KTI // 2):
                    pt8, jj = (
                        (ptl_tiles[qc], j)
                        if j < HKT // 2
                        else (ptp_tiles[qc], j - HKT // 2)
                    )
                    nc.tensor.matmul(
                        rs_ps,
                        lhsT=ones8,
                        rhs=pt8[:, 2 * jj : 2 * jj + 2, :],
                        start=(j == 0),
                        stop=(j == KTI // 2 - 1),
                        perf_mode=DR,
                    )
                # 256*Z = 2^20 + 16*rs  (rs = 16 * sum_k p)
                zt = outp.tile([P, NQ], f32, tag="zt", bufs=2)
                nc.vector.tensor_scalar(
                    out=zt,
                    in0=rs_ps,
                    scalar1=16.0,
                    scalar2=float(1 << 20),
                    op0=mybir.AluOpType.mult,
                    op1=mybir.AluOpType.add,
                )
                recip = outp.tile([P, NQ], f32, tag="recip", bufs=2)
                nc.vector.reciprocal(recip, zt)
                return recip

            def ot_group(qc, e, recip):
                qsl = slice(qc * NQ, (qc + 1) * NQ)
                ops = ps_ot.tile([P, NQ], f32, tag="ot")
                for j in range(KTI // 2):
                    if j < HKT // 2:
                        vt, pt8, jj = v_loc, ptl_tiles[qc], j
                    else:
                        vt, pt8, jj = v_rem, ptp_tiles[qc], j - HKT // 2
                    nc.tensor.matmul(
                        ops,
                        lhsT=vt[:, 2 * jj : 2 * jj + 2, e * P : (e + 1) * P],
                        rhs=pt8[:, 2 * jj : 2 * jj + 2, :],
                        start=(j == 0),
                        stop=(j == KTI // 2 - 1),
                        perf_mode=DR,
                    )
                tmp = outp.tile([P, NQ], f32, tag="tmp", bufs=3)
                # tmp = (ops + colsum_e) * (1/Z); then + bv, all off the
                # Scalar engine (which is saturated by the exp stream).
                nc.vector.scalar_tensor_tensor(
                    out=tmp,
                    in0=ops,
                    scalar=colsum_sb[:, e : e + 1],
                    in1=recip,
                    op0=mybir.AluOpType.add,
                    op1=mybir.AluOpType.mult,
                )
                nc.vector.tensor_scalar(
                    out=tmp,
                    in0=tmp,
                    scalar1=bv_sb[:, e : e + 1],
                    scalar2=None,
                    op0=mybir.AluOpType.add,
                )
                nc.sync.dma_start(out=ot[e * P : (e + 1) * P, qsl], in_=tmp)

            def finish_pieces(qc):
                # finish(qc) as filler closures: the Z pass, then one OT
                # accumulation group per e-tile. `recip` is created by the
                # Z closure and captured by reference for the OT ones.
                box = {}

                def zp():
                    box["recip"] = z_group(qc)

                return [zp] + [
                    (lambda e=e: ot_group(qc, e, box["recip"])) for e in range(ET)
                ]

            def finish(qc):
                for f in finish_pieces(qc):
                    f()

            # Local-half score blocks with the V projection (and later QT
            # chunks) interleaved as PE fillers; then the usual staggered
            # remote-half + finish sequence.
            qt_group(0, on_scalar=True)
            st_block(0, 0, HKT, fillers=[lambda k=k: v_group(k) for k in range(8)])
            qt_group(1, on_scalar=False)
            st_block(
                1, 0, HKT,
                fillers=[lambda k=k: v_group(k) for k in range(8, HKT)]
                + [v_exchange],
            )
            qt_group(2, on_scalar=False)
            st_block(2, 0, HKT)
            qt_group(3, on_scalar=False)
            st_block(3, 0, HKT)
            st_block(0, HKT, KTI)
            st_block(1, HKT, KTI, fillers=finish_pieces(0))
            st_block(2, HKT, KTI, fillers=finish_pieces(1))
            st_block(3, HKT, KTI, fillers=finish_pieces(2))
            finish(3)

    _split_excess_waits(nc, mybir)
    return nc


def _get_nc():
    if "nc" not in _CACHE:
        _CACHE["nc"] = _build_nc()
    return _CACHE["nc"]


def _make_in_maps(x, Wq, bq, Wk, bk, Wv, bv):
    bf16 = ml_dtypes.bfloat16
    # x16 lifts Q/K/V (and hence fp8-stored operands) out of the e4m3
    # subnormal range; the kernel folds every factor back out exactly.
    wqT = np.ascontiguousarray(Wq.T * 16.0).astype(bf16)
    wkT = np.ascontiguousarray(Wk.T * 16.0).astype(bf16)
    wvT = np.ascontiguousarray(Wv.T * 16.0).astype(bf16)
    bqp = np.ascontiguousarray(bq.reshape(ET, P).T * 16.0).astype(np.float32)
    bkp = np.ascontiguousarray(bk.reshape(ET, P).T * 16.0).astype(np.float32)
    bvp = np.ascontiguousarray(bv.reshape(ET, P).T).astype(np.float32)
    in_maps = []
    for c in range(NCORES):
        b, h = divmod(c, 2)
        # Local half of x[b].T: both this core's query columns and its K/V
        # half (they are the same row range by construction).
        xTl = np.ascontiguousarray(x[b, h * SQ : (h + 1) * SQ, :].T).astype(bf16)
        # Partner-half row bases into the rank-ordered AllGather outputs.
        poff = np.array(
            [[(1 - h) * ET * P, (1 - h) * HKT * P]], dtype=np.uint32
        )
        in_maps.append(
            {
                "xT": xTl,
                "poff": poff,
                "wqT": wqT,
                "wkT": wkT,
                "wvT": wvT,
                "bq": bqp,
                "bk": bkp,
                "bv": bvp,
            }
        )
    return in_maps


def _run(in_maps, **kwargs):
    from concourse.bass_utils import run_bass_kernel_spmd

    nc = _get_nc()
    return run_bass_kernel_spmd(nc, in_maps, core_ids=list(range(NCORES)), **kwargs)


def kernel(x, Wq, bq, Wk, bk, Wv, bv):
    x = np.asarray(x, dtype=np.float32)
    Wq = np.asarray(Wq, dtype=np.float32)
    Wk = np.asarray(Wk, dtype=np.float32)
    Wv = np.asarray(Wv, dtype=np.float32)
    bq = np.asarray(bq, dtype=np.float32)
    bk = np.asarray(bk, dtype=np.float32)
    bv = np.asarray(bv, dtype=np.float32)

    res = _run(_make_in_maps(x, Wq, bq, Wk, bk, Wv, bv))
    out = np.empty((B, S, D), dtype=np.float32)
    for c in range(NCORES):
        b, h = divmod(c, 2)
        out[b, h * SQ : (h + 1) * SQ, :] = np.asarray(res.results[c]["ot"]).T
    return out
